# revision 6
# baseline (speedup 1.0000x reference)
"""Trainium2 Bass kernel for the DependencyAnalyzer GNN problem.

Computation (reference semantics):
    h = relu(features @ W_node + b_node)                  # [N, H]
    2x: agg = scatter_add(h[src] -> dst);  h = relu((h + agg) @ W_conv + b_conv)
    out = stack([ (m*h) @ (m*h).T,  h @ h.T ])            # m = (nodes == 2)

Strategy (8 NeuronCores, SPMD):
  - Host reformats the edge list into per-core dense adjacency blocks
    A'^T [src=8192, dst_local=1024] in fp8 (counts are exact), with the
    identity folded in (A' = A + I_c) so that A' @ h == h_block + agg.
  - h is fp16 end-to-end (validated: 3.6e-3 max rel err vs the 2e-2
    gate): every core computes h0 for all nodes (replicated); round
    matmuls use fp16 h (stationary) against fp8 A (moving).
  - Round 1 output is exchanged via two fp16 AllGathers; round 2 starts
    on the AG1a half while AG1b flies.
  - Both outputs are symmetric and function_deps = mask.outer * sim, so
    the device computes ONLY the upper triangle of sim: a uniform
    17-cell-per-core cover of the 136 upper [512x512] cells. Stationary
    is always the core's own h strip; the other strip comes from a
    per-core ROTATED gather out of the final AllGather (dynamic-offset
    pair DMAs driven by an index input), so the instruction stream is
    identical across cores. Cells run as even/odd tile_position pairs.
  - sim cells are written as bf16; the host casts, mirrors, and applies
    the fdeps mask during output assembly.

Latency schedule (v2): a dummy collective at t~12us absorbs the ~20us
first-collective CC-engine warmup; features/W land before constants so
phase 1 starts at ~13us; phase-1 matmuls run as concurrent 2-row-tile
pairs; round tails evacuate PSUM on vector+scalar in parallel and
trigger each AllGather half as soon as its input DMA is issued; the
post-AG1 re-gather is one descriptor per half; phase-3 output stores
all ride the sync queue while the rotated strip gathers own the scalar
queue in first-needed order.
"""

import numpy as np
import ml_dtypes

import concourse.bass as bass
import concourse.mybir as mybir
import concourse.tile as tile
from concourse import masks
from concourse.bass import DynSlice
from concourse.bass_utils import run_bass_kernel_spmd

N = 8192          # nodes
NB = 1024         # nodes per core block
NCORES = 8
F = 10            # feature dim
FA = F + 1        # +1 ones row (bias fold)
H = 64            # hidden dim
KT = N // 128     # 64 src k-tiles
MT = NB // 128    # 8 own m-tiles
F32 = mybir.dt.float32
F16 = mybir.dt.float16
BF16 = mybir.dt.bfloat16
F8 = mybir.dt.float8e4
I32 = mybir.dt.int32
RELU = mybir.ActivationFunctionType.Relu

# ---- the 19-cell symmetric cover -----------------------------------------
# cell = (sigma, rho): sim[own strip sigma (512 rows)] x [rot strip rho],
# rot strip rho = absolute strip (2c + rho) % 16 (pure rotation).  rho 0,1
# are the core's own strips.  The distance-4 block pair is covered twice
# (both partner cores compute all four of its cells) so the instruction
# stream stays core-uniform.  Cells run as tile_position pairs: one matmul
# on PE rows 0:64 (operands at partitions 0:64), one on rows 64:128.
# Gathered strip rho sits at partition base 64*((rho//2) % 2), column slot
# (rho//2)-1 for evens / 6+(rho//2) for odds of the rhs tile.
# Schedule per sigma: "own" runs before the final AllGathers, "even" after
# AG2a (even strips), "odd" after AG2b.  Pairs are (rho@base0, rho@base64).
SCHED = {
    0: {"own": [(0, 1)], "even": [(4, 2), (8, 6)], "odd": [(9, 11), (13, 15)]},
    1: {"own": [(None, 1)], "even": [(12, 10), (8, 14)], "odd": [(5, 3), (9, 7)]},
}
# output column slot (x512) in out_ext for each (sigma, rho) cell
OUT_SLOT = {
    (0, 0): 0, (0, 1): 1, (0, 4): 2, (0, 2): 3, (0, 8): 4, (0, 6): 5,
    (0, 9): 6, (0, 11): 7, (0, 13): 8, (0, 15): 9,
    (1, 1): 0, (1, 12): 1, (1, 10): 2, (1, 8): 3, (1, 14): 4,
    (1, 5): 5, (1, 3): 6, (1, 9): 7, (1, 7): 8,
}
NSLOT = {0: 10, 1: 9}
# rotated-strip gather issue order = first-needed order in the tau loop
EVEN_RHO_ORDER = [4, 2, 8, 6, 12, 10, 14]
ODD_RHO_ORDER = [9, 11, 13, 15, 5, 3, 7]


def rot_table(c):
    """Absolute 512-strip index for each rotated slot rho of core c."""
    return [(2 * c + r) % 16 for r in range(16)]


LAST_RESULT = None  # BassKernelResults of the most recent run (for test harness)


def _ensure_trace_hook():
    """Best-effort: register the NTFF profiling hook for trace=True runs."""
    import sys as _sys
    import types as _types

    try:
        if "antenv.axon_hooks" in _sys.modules:
            return
        import antenv as _antenv

        mod = _types.ModuleType("antenv.axon_hooks")
        _state = {"hook": None}
        mod.set_axon_ntff_profile_hook = lambda h: _state.__setitem__("hook", h)
        mod.get_axon_ntff_profile_hook = lambda: _state["hook"]
        _sys.modules["antenv.axon_hooks"] = mod
        _antenv.axon_hooks = mod

        from trn_agent_boot.trn_boot import _ntff_profile_via_ctypes

        so_path = "/opt/axon/libaxon_pjrt.so"
        import os as _os

        if _os.path.exists(so_path):
            hook = _ntff_profile_via_ctypes(so_path)
            if hook is not None:
                mod.set_axon_ntff_profile_hook(hook)
    except Exception:
        pass


def _legalize_waits(nc, max_waits=1):
    """This walrus build accepts at most one sync-wait per lowered HW
    instruction; hoist extra waits onto standalone EventSemaphore
    instructions on the same (in-order) engine queue."""
    n_fixed = 0
    for f in nc.m.functions:
        for bb in f.blocks:
            new_list = []
            for ins in bb.instructions:
                si = ins.sync_info
                if si is not None and len(si.on_wait) > max_waits:
                    waits = list(si.on_wait)
                    for w in waits[: len(waits) - max_waits]:
                        ev = mybir.InstEventSemaphore(
                            name=f"{ins.name}-w-{w.ant_name}",
                            ins=[],
                            outs=[],
                            sync_info=mybir.SyncInfo(on_wait=[w], on_update=[]),
                            engine=ins.engine,
                        )
                        new_list.append(ev)
                    ins.sync_info = mybir.SyncInfo(
                        on_wait=waits[len(waits) - max_waits :],
                        on_update=list(si.on_update),
                    )
                    n_fixed += 1
                new_list.append(ins)
            bb.instructions = new_list
    return n_fixed


def _build_nc():
    nc = bass.Bass(num_devices=NCORES)

    # ---- external I/O (same program on all cores; per-core data differs) ----
    # features^T split by k-tile parity: featEv holds even chunks (128 cols
    # each) of both halves, featOd the odd chunks -- so each SBUF feature
    # tile carries even chunks at partitions 0:33 and odd at 64:97 for
    # concurrent 2-row-tile phase-1 matmul pairs.
    featEv = nc.declare_dram_parameter("featEv", [3 * FA, N // 2], BF16, isOutput=False)
    featOd = nc.declare_dram_parameter("featOd", [3 * FA, N // 2], BF16, isOutput=False)
    WnA = nc.declare_dram_parameter("W3", [3 * FA, H], BF16, isOutput=False)
    Wc16 = nc.declare_dram_parameter("Wc16", [H, H], F16, isOutput=False)
    bc = nc.declare_dram_parameter("bc", [H, 1], F32, isOutput=False)
    rot_idx = nc.declare_dram_parameter("rot_idx", [1, 7], I32, isOutput=False)
    # A'^T p-major: A_p[p, k*1024 + n] = A'^T[k*128 + p, n], fp8 counts
    A_p = nc.declare_dram_parameter("A_p", [128, KT * NB], F8, isOutput=False)
    # out[tau*128+p, slot*512 + f]: sim cell values (see OUT_SLOT)
    out_ext = nc.declare_dram_parameter("out", [NB, 10 * 512], BF16, isOutput=True)

    # ---- internal DRAM (collective bounce buffers) ----
    warm_in = nc.dram_tensor("warm_in", [1, 8], BF16)
    warm_out = nc.dram_tensor("warm_out", [NCORES, 8], BF16, addr_space="Shared")
    ag1a_in = nc.dram_tensor("ag1a_in", [NB // 2, H], F16)
    ag1a_out = nc.dram_tensor("ag1a_out", [N // 2, H], F16, addr_space="Shared")
    ag1b_in = nc.dram_tensor("ag1b_in", [NB // 2, H], F16)
    ag1b_out = nc.dram_tensor("ag1b_out", [N // 2, H], F16, addr_space="Shared")
    # final h, fp16: AG2a carries every core's even strip (local cols 0:512,
    # T layout), AG2b the odd strip; out row r*64+k = strip-of-rank-r row k
    ag2a_in = nc.dram_tensor("ag2a_in", [H, 512], F16)
    ag2a_out = nc.dram_tensor("ag2a_out", [8 * H, 512], F16, addr_space="Shared")
    ag2b_in = nc.dram_tensor("ag2b_in", [H, 512], F16)
    ag2b_out = nc.dram_tensor("ag2b_out", [8 * H, 512], F16, addr_space="Shared")
    rg = [list(range(NCORES))]

    with tile.TileContext(nc, num_cores=NCORES) as tc:
        with tc.tile_pool(name="persist", bufs=1) as persist:
            # Dummy collective FIRST: pays the ~20us cold-start of the CC
            # engine in the shadow of the input DMAs so AG1a begins ~1us
            # after its trigger.  Collectives can't read IO tensors, so
            # bounce 16 bytes of the zeroed dummy tile through internal
            # DRAM (ready by ~13us; AG1a isn't until ~40us).
            warm_s = persist.tile([1, 8], BF16)
            nc.vector.memset(warm_s[:], 0.0)
            nc.sync.dma_start(out=warm_in[:], in_=warm_s[:])
            nc.gpsimd.collective_compute(
                "AllGather",
                mybir.AluOpType.bypass,
                replica_groups=rg,
                ins=[warm_in[:]],
                outs=[warm_out[:]],
            )

            # ---------------- constants / small inputs ----------------------
            # sync queue: W then features (phase-1 critical path), then half
            # the A tiles.  scalar queue: everything else + other A half.
            wn_s = persist.tile([64 + 3 * FA, H], BF16)
            nc.sync.dma_start(out=wn_s[0 : 3 * FA, :], in_=WnA[:])
            nc.sync.dma_start(out=wn_s[64 : 64 + 3 * FA, :], in_=WnA[:])
            # W_conv on both partition halves so the two dst-half W matmuls
            # can run as a tile_position row-group pair
            wc_s = persist.tile([128, H], F16)
            nc.scalar.dma_start(out=wc_s[0:H, :], in_=Wc16[:])
            nc.scalar.dma_start(out=wc_s[H:128, :], in_=Wc16[:])
            bc_s = persist.tile([H, 1], F32)
            nc.scalar.dma_start(out=bc_s[:], in_=bc[:])
            rot_s = persist.tile([1, 7], I32)
            nc.scalar.dma_start(out=rot_s[:], in_=rot_idx[:])
            ident = persist.tile([H, H], F16)
            masks.make_identity(nc, ident[:])
            dummy_s = persist.tile([1, 512], BF16)
            nc.vector.memset(dummy_s[:], 0.0)

            # rotation indices (c+k)%8, k=1..7 -> registers for the per-core
            # rotated gathers out of the two final AllGathers
            rot_vals = [
                nc.values_load(
                    rot_s[0:1, i : i + 1],
                    min_val=0,
                    max_val=7,
                    skip_runtime_bounds_check=True,
                )
                for i in range(7)
            ]

            def absorb(pt, parts, free):
                # Dummy full-tile matmul: soaks up PSUM pool-boundary WAR
                # waits on PE so real matmuls stay within the ISA's sync
                # wait budget.
                nc.tensor.matmul(
                    pt[:, :],
                    dummy_s[0:1, 0:parts],
                    dummy_s[0:1, 0:free],
                    start=True,
                    stop=True,
                )

            # final h (own block, T layout, fp16), duplicated on partitions
            # 64:128 for tile_position-paired K=64 matmuls in phase 3
            hT16d = persist.tile([128, NB], F16)

            with (
                tc.tile_pool(name="apool", bufs=16) as apool,
                tc.tile_pool(name="hpool", bufs=KT) as hpool,
            ):
                # ------------- phase 1: h0 for all nodes (replicated) -------
                # Concurrent row-tile pairs: even k-chunk at partitions 0:33
                # (tile (0,0)), odd at 64:97 (tile (64,0)).
                h0_tiles = [None] * KT
                with (
                    tc.tile_pool(name="ph1", bufs=2) as ph1,
                    tc.tile_pool(name="pp1", bufs=4, space="PSUM") as pp1,
                ):
                    ft_halves = []
                    for half in range(2):
                        ft_h = ph1.tile(
                            [64 + 3 * FA, N // 4], BF16, tag=f"ft{half}", bufs=1
                        )
                        nc.sync.dma_start(
                            out=ft_h[0 : 3 * FA, :],
                            in_=featEv[:, half * (N // 4) : (half + 1) * (N // 4)],
                        )
                        nc.sync.dma_start(
                            out=ft_h[64 : 64 + 3 * FA, :],
                            in_=featOd[:, half * (N // 4) : (half + 1) * (N // 4)],
                        )
                        ft_halves.append(ft_h)

                    # adjacency, fp8, resident in SBUF for both rounds;
                    # alternate queues so descriptor gen is 2-wide
                    a_tiles = []
                    for j in range(16):
                        at = apool.tile([128, 4 * NB], F8, name=f"a{j}", tag="A")
                        eng = nc.sync if j % 2 == 0 else nc.scalar
                        eng.dma_start(
                            out=at[:], in_=A_p[:, j * 4 * NB : (j + 1) * 4 * NB]
                        )
                        a_tiles.append(at)

                    def a_slice(k, nh):
                        t = a_tiles[k // 4]
                        off = (k % 4) * NB + nh * 512
                        return t[:, off : off + 512]

                    first_p1 = True
                    for half in range(2):
                        ft_h = ft_halves[half]
                        for j in range(KT // 4):  # 16 pairs per half
                            csl = slice(j * 128, (j + 1) * 128)
                            for par, pbase in ((0, 0), (1, 64)):
                                k = half * (KT // 2) + 2 * j + par
                                ps = pp1.tile([128, H], F32, tag="p64", bufs=4)
                                if first_p1:
                                    absorb(ps, 128, H)
                                    first_p1 = False
                                nc.tensor.matmul(
                                    ps[:],
                                    ft_h[pbase : pbase + 3 * FA, csl],
                                    wn_s[pbase : pbase + 3 * FA, :],
                                    start=True,
                                    stop=True,
                                    tile_position=(pbase, 0),
                                    skip_group_check=True,
                                )
                                hl = hpool.tile([128, H], F16, name=f"h0_{k}", tag="HL")
                                nc.scalar.activation(hl[:], ps[:], RELU)
                                h0_tiles[k] = hl

                # ------------- phase 2: two message-passing rounds ----------
                cur_tiles = h0_tiles
                rnd2_korder = list(range(KT))
                for rnd in (1, 2):
                    with (
                        tc.tile_pool(name=f"rd{rnd}", bufs=1) as rd,
                        tc.tile_pool(name=f"prd{rnd}", bufs=1, space="PSUM") as prd,
                    ):
                        # both dst halves accumulate in ONE [128, 512] psum:
                        # half nh at partitions nh*64, via tile_position
                        # column-groups -- the two M=64 matmuls of each
                        # k-tile run CONCURRENTLY on the PE array
                        psaP = prd.tile([128, 512], F32, tag="psaP")
                        aggP = rd.tile([128, 512], F16, tag="aggP", bufs=2)
                        if rnd == 1:
                            absorb(psaP, 128, 512)
                            hT16 = rd.tile([H, NB], F16, tag="hT16r1")
                            nrm_t = [
                                rd.tile(
                                    [128, 4 * H], F16, tag=f"nrm{hf}",
                                    name=f"nrm{hf}",
                                )
                                for hf in range(2)
                            ]

                        ks = list(range(KT)) if rnd == 1 else rnd2_korder
                        for ki, k in enumerate(ks):
                            for nh in (0, 1):
                                nc.tensor.matmul(
                                    psaP[nh * H : (nh + 1) * H, :],
                                    cur_tiles[k],
                                    a_slice(k, nh),
                                    start=(ki == 0),
                                    stop=(ki == KT - 1),
                                    tile_position=(0, nh * H),
                                    skip_group_check=True,
                                )

                        # PSUM evacuation split across vector+scalar (runs
                        # in parallel); then per dst half: W matmul, act,
                        # input DMA, collective trigger -- each AllGather
                        # half fires as early as possible.
                        nc.vector.tensor_copy(aggP[:, 0:256], psaP[:, 0:256])
                        nc.scalar.copy(aggP[:, 256:512], psaP[:, 256:512])

                        for nh in (0, 1):
                            hsl = slice(nh * H, (nh + 1) * H)
                            nsl = slice(nh * 512, (nh + 1) * 512)
                            psw = prd.tile([H, 512], F32, tag="psw", bufs=2)
                            if nh == 0 and rnd == 1:
                                absorb(psw, H, 512)
                            # W matmuls pair as a K row-group (0,0)/(64,0)
                            nc.tensor.matmul(
                                psw[:],
                                wc_s[hsl, :],
                                aggP[hsl, :],
                                start=True,
                                stop=True,
                                tile_position=(nh * H, 0),
                            )
                            if rnd == 1:
                                # act: scalar for half0, vector for half1
                                # (relu(x + b) as tensor_scalar add+max)
                                if nh == 0:
                                    nc.scalar.activation(
                                        hT16[:, nsl], psw[:], RELU, bias=bc_s[:]
                                    )
                                else:
                                    nc.vector.tensor_scalar(
                                        hT16[:, nsl],
                                        psw[:],
                                        bc_s[:],
                                        0.0,
                                        mybir.AluOpType.add,
                                        mybir.AluOpType.max,
                                    )
                                # transpose this half's 4 m-tiles into one
                                # staging tile; single DMA feeds AG1{a,b}
                                agi, ago = (
                                    (ag1a_in, ag1a_out) if nh == 0
                                    else (ag1b_in, ag1b_out)
                                )
                                nrm = nrm_t[nh]
                                for mm in range(MT // 2):
                                    m = nh * (MT // 2) + mm
                                    pst = prd.tile([128, H], F16, tag="pst", bufs=2)
                                    nc.tensor.transpose(
                                        pst[:],
                                        hT16[:, m * 128 : (m + 1) * 128],
                                        ident[:],
                                    )
                                    if mm % 2 == 0:
                                        nc.vector.tensor_copy(
                                            nrm[:, mm * H : (mm + 1) * H], pst[:]
                                        )
                                    else:
                                        nc.scalar.copy(
                                            nrm[:, mm * H : (mm + 1) * H], pst[:]
                                        )
                                nc.sync.dma_start(
                                    out=agi[:].rearrange("(t p) c -> p t c", p=128),
                                    in_=nrm[:].rearrange("p (t c) -> p t c", t=4),
                                )
                            else:
                                if nh == 0:
                                    nc.scalar.activation(
                                        hT16d[0:H, nsl], psw[:], RELU, bias=bc_s[:]
                                    )
                                else:
                                    nc.vector.tensor_scalar(
                                        hT16d[0:H, nsl],
                                        psw[:],
                                        bc_s[:],
                                        0.0,
                                        mybir.AluOpType.add,
                                        mybir.AluOpType.max,
                                    )
                                agi, ago = (
                                    (ag2a_in, ag2a_out) if nh == 0
                                    else (ag2b_in, ag2b_out)
                                )
                                nc.sync.dma_start(out=agi[:], in_=hT16d[0:H, nsl])
                            nc.gpsimd.collective_compute(
                                "AllGather",
                                mybir.AluOpType.bypass,
                                replica_groups=rg,
                                ins=[agi[:]],
                                outs=[ago[:]],
                            )

                        if rnd == 1:
                            # round-2 operands come from the gathered halves
                            # (own-block k is core-dependent, so the local
                            # nrm tiles can't be referenced uniformly);
                            # one descriptor per half: [4096, 64] ->
                            # [128 p, 8 g, 4 t, 64]
                            cur_tiles = [None] * KT
                            korder = []
                            for half, ago in [(0, ag1a_out), (1, ag1b_out)]:
                                hl8 = hpool.tile(
                                    [128, 32 * H], F16,
                                    name=f"h1_{half}", tag="HL8", bufs=2,
                                )
                                eng = nc.sync if half == 0 else nc.scalar
                                eng.dma_start(
                                    out=hl8[:].rearrange(
                                        "p (g t c) -> p g t c", g=8, t=4
                                    ),
                                    in_=ago[:].rearrange(
                                        "(g t p) c -> p g t c", p=128, t=4
                                    ),
                                )
                                for g in range(8):
                                    for t in range(4):
                                        k = g * 8 + half * 4 + t
                                        off = (g * 4 + t) * H
                                        cur_tiles[k] = hl8[:, off : off + H]
                                        korder.append(k)
                            rnd2_korder = korder
                        else:
                            # duplicate final h to partitions 64:128 for the
                            # tile_position-paired matmuls
                            nc.sync.dma_start(
                                out=hT16d[H:128, :], in_=hT16d[0:H, :]
                            )

            # ---------------- phase 3: sim upper cells + output -------------
            # 17 [512x512] cells as even/odd tile_position pairs; stationary
            # = own h strip (hT16d), moving = rotated strips in rhs2:
            # slot k partitions 0:64 = strip 2k, 64:128 = strip 2k+1.
            with (
                tc.tile_pool(name="ph3", bufs=1) as ph3,
                tc.tile_pool(name="stg", bufs=1) as stg,
                tc.tile_pool(name="pp3", bufs=8, space="PSUM") as pp3,
            ):
                rhs2 = ph3.tile([128, 14 * 512], F16, tag="rhs2")

                def rbase(rho):
                    # partition base of gathered strip rho (see header)
                    return H * ((rho // 2) % 2)

                def rcol(rho):
                    return (rho // 2) - 1 if rho % 2 == 0 else 6 + rho // 2

                def issue_gathers():
                    # rotated gather, all on the scalar queue (the sync
                    # queue carries every output store), in first-needed
                    # order: even strips wait on AG2a, odd on AG2b.
                    for rho in EVEN_RHO_ORDER + ODD_RHO_ORDER:
                        k = rho // 2
                        v = rot_vals[k - 1]
                        src = ag2a_out if rho % 2 == 0 else ag2b_out
                        nc.scalar.dma_start(
                            out=rhs2[
                                rbase(rho) : rbase(rho) + H,
                                rcol(rho) * 512 : (rcol(rho) + 1) * 512,
                            ],
                            in_=src[DynSlice(v * H, H), :],
                        )

                def mov(rho):
                    # moving operand of cell rho; own strips from hT16d
                    if rho == 0:
                        return hT16d[0:H, 0:512]
                    if rho == 1:
                        return hT16d[H:128, 512:1024]
                    b = rbase(rho)
                    return rhs2[b : b + H, rcol(rho) * 512 : (rcol(rho) + 1) * 512]

                # phase-contiguous out columns: own slots [0, ow), even
                # [ow, ow+4), odd [ow+4, ow+8), ow = 2 (sigma 0) / 1 (sigma 1)
                OWN_W = {0: 2, 1: 1}
                first = True
                ncopy = 0
                for phase in ("own", "even", "odd"):
                    if phase == "even":
                        issue_gathers()
                    for tau in range(8):
                        sigma, mt = tau // 4, tau % 4
                        chunk = slice(
                            sigma * 512 + mt * 128, sigma * 512 + (mt + 1) * 128
                        )
                        ow = OWN_W[sigma]
                        slot0 = {"own": 0, "even": ow, "odd": ow + 4}[phase]
                        nsl = OWN_W[sigma] if phase == "own" else 4
                        stA = stg.tile(
                            [128, 4 * 512], BF16, tag=f"st_{phase}", bufs=4
                        )
                        for rho0, rho64 in SCHED[sigma][phase]:
                            for rho, pbase in ((rho0, 0), (rho64, H)):
                                if rho is None:
                                    continue
                                ps3 = pp3.tile([128, 512], F32, tag="ps3", bufs=8)
                                if first:
                                    absorb(ps3, 128, 512)
                                    first = False
                                nc.tensor.matmul(
                                    ps3[:],
                                    hT16d[pbase : pbase + H, chunk],
                                    mov(rho),
                                    start=True,
                                    stop=True,
                                    tile_position=(pbase, 0),
                                )
                                slot = OUT_SLOT[(sigma, rho)] - slot0
                                dst = stA[:, slot * 512 : (slot + 1) * 512]
                                if ncopy % 2 == 0:
                                    nc.scalar.copy(dst, ps3[:])
                                else:
                                    nc.vector.tensor_copy(dst, ps3[:])
                                ncopy += 1
                        rsl = slice(tau * 128, (tau + 1) * 128)
                        nc.sync.dma_start(
                            out=out_ext[rsl, slot0 * 512 : (slot0 + nsl) * 512],
                            in_=stA[:, 0 : nsl * 512],
                        )
    _legalize_waits(nc)
    return nc


def _host_prep(features, W_node, b_node, W_conv, b_conv, nodes, edges):
    features = np.asarray(features, np.float32)
    W_node = np.asarray(W_node, np.float32)
    b_node = np.asarray(b_node, np.float32)
    W_conv = np.asarray(W_conv, np.float32)
    b_conv = np.asarray(b_conv, np.float32)
    edges = np.asarray(edges)

    def _hilo(x):
        hi = x.astype(ml_dtypes.bfloat16)
        lo = (x - hi.astype(np.float32)).astype(ml_dtypes.bfloat16)
        return hi, lo

    # [features.T; ones] and [W_node; b_node], K-stacked for bf16 hi/lo:
    # [fa_hi; fa_lo_z; fa_hi] . [Wa_hi; Wa_hi; Wa_lo] ~= f@W + b
    fa = np.concatenate([features.T, np.ones((1, N), np.float32)], axis=0)
    Wa = np.concatenate([W_node, b_node[None, :]], axis=0)
    fa_hi, fa_lo = _hilo(fa)
    fa_lo_z = fa_lo.copy()
    fa_lo_z[F, :] = 0  # no double-counted bias
    Wa_hi, Wa_lo = _hilo(Wa)
    featT3 = np.concatenate([fa_hi, fa_lo_z, fa_hi], axis=0)  # [33, N] bf16
    W3 = np.concatenate([Wa_hi, Wa_hi, Wa_lo], axis=0)  # [33, H] bf16

    # split into even / odd 128-col k-chunks (see _build_nc phase 1)
    ft4 = featT3.reshape(3 * FA, KT, 128)
    featEv = np.ascontiguousarray(ft4[:, 0::2, :].reshape(3 * FA, N // 2))
    featOd = np.ascontiguousarray(ft4[:, 1::2, :].reshape(3 * FA, N // 2))

    src = edges[:, 0].astype(np.int64)
    dst = edges[:, 1].astype(np.int64)
    in_maps = []
    for c in range(NCORES):
        sel = (dst >= c * NB) & (dst < (c + 1) * NB)
        idx = src[sel] * NB + (dst[sel] - c * NB)
        cnt = np.bincount(idx, minlength=N * NB).astype(np.float32).reshape(N, NB)
        cnt[c * NB + np.arange(NB), np.arange(NB)] += 1.0  # fold identity
        assert cnt.max() <= 16, "adjacency counts exceed exact fp8 range"
        A_pm = np.ascontiguousarray(
            cnt.reshape(KT, 128, NB).transpose(1, 0, 2).reshape(128, KT * NB)
        ).astype(ml_dtypes.float8_e4m3)
        in_maps.append(
            {
                "featEv": featEv,
                "featOd": featOd,
                "W3": W3,
                "Wc16": W_conv.astype(np.float16),
                "bc": b_conv.reshape(H, 1),
                "rot_idx": np.asarray(
                    [(c + k) % 8 for k in range(1, 8)], np.int32
                )[None, :],
                "A_p": A_pm,
            }
        )
    return in_maps


def _assemble(results, nodes):
    """Scatter per-core sim cells into [2, N, N] fp32; mirror and mask."""
    out = np.empty((2, N, N), np.float32)
    sim = out[1]
    for c in range(NCORES):
        T = rot_table(c)
        o = np.asarray(results[c]["out"]).astype(np.float32)  # [1024, 5120]
        for (sigma, rho), slot in OUT_SLOT.items():
            i, j = 2 * c + sigma, T[rho]
            B = o[sigma * 512 : (sigma + 1) * 512, slot * 512 : (slot + 1) * 512]
            sim[i * 512 : (i + 1) * 512, j * 512 : (j + 1) * 512] = B
            if i != j:
                sim[j * 512 : (j + 1) * 512, i * 512 : (i + 1) * 512] = B.T
    m = (np.asarray(nodes) == 2).astype(np.float32)
    np.multiply(sim, m[:, None], out=out[0])
    np.multiply(out[0], m[None, :], out=out[0])
    return out


def kernel(features, W_node, b_node, W_conv, b_conv, nodes, edges, **kw):
    global LAST_RESULT
    _ensure_trace_hook()
    in_maps = _host_prep(features, W_node, b_node, W_conv, b_conv, nodes, edges)
    nc = _build_nc()
    res = run_bass_kernel_spmd(nc, in_maps, core_ids=list(range(NCORES)))
    LAST_RESULT = res
    return _assemble(res.results, nodes)


if __name__ == "__main__":
    np.random.seed(0)
    feats = np.random.randn(N, F).astype(np.float32)
    ins = {
        "features": feats,
        "W_node": (np.random.randn(F, H) * 0.1).astype(np.float32),
        "b_node": (np.random.randn(H) * 0.1).astype(np.float32),
        "W_conv": (np.random.randn(H, H) * 0.05).astype(np.float32),
        "b_conv": (np.random.randn(H) * 0.05).astype(np.float32),
        "nodes": np.random.randint(0, 5, N, dtype=np.int32),
        "edges": np.random.randint(0, N, (524288, 2), dtype=np.int32),
    }
    out = kernel(**ins)
    print(out.shape, out.dtype)
